# revision 37
# baseline (speedup 1.0000x reference)
"""Trainium2 Bass kernel for the AESINDy dense-MLP problem.

Computation (per row of bin0 [B=262144, 256]):
  encoder MLP 256 ->(relu) 128 ->(relu) 64 ->(relu) 32 -> 8
  z9 = concat(z, M)                                  # 9 latent dims
  lib = polynomial library of z9 up to order 3       # 220 terms
  out = (lib @ sindy_w.T)[:, :8]                     # [B, 1, 8]

Strategy: pure data parallel over 8 NeuronCores (B/8 = 32768 rows each).
On-chip dataflow is feature-major: input tiles are transposed on the
TensorEngine (regular matmuls against an identity), then the whole MLP is a
chain of small matmuls with weights as the stationary operand.  The SINDy
polynomial library is never materialized: with selection matrices
(host-precomputed from sindy_w) the quadratic/cubic terms reduce to
  F  = SAB @ z9          (one matmul, factor gather)
  P2 = F[:45] * F[45:]   (elementwise, all 45 pair products)
  H  = Astack @ P2       (per-leading-index cubic partial sums)
  ZH = (REP @ z9) * H    (elementwise)
  out = CQ @ P2 + CL @ z9 + RED^T @ ZH + c0   (PSUM accumulation)

kernel(**inputs) takes the full unsharded inputs and returns the full
[B, 1, 8] float32 output.
"""
import numpy as np
from itertools import combinations_with_replacement as cwr
from contextlib import ExitStack

import concourse.bass as bass
import concourse.tile as tile
from concourse.tile import add_dep_helper
from concourse import mybir
from concourse.bass_utils import run_bass_kernel_spmd
from concourse.masks import make_identity

# problem constants (hardcoded per spec nn_AESINDy_3753801416885)
B = 262144
N_CORES = 8
BC = B // N_CORES            # 32768 rows per core
NBINS = 256
DIMS = [256, 128, 64, 32, 8]
D = 9                        # D_SINDY
NQ = 45                      # quadratic terms
NT = 165                     # cubic terms
L = 220                      # library size

SUPER = 2048                 # batch rows per DMA super-tile
NSUP = BC // SUPER           # 16 super-tiles
TN = 512                     # matmul moving free dim (one PSUM bank of fp32)
NTILE = SUPER // TN          # 4 inner tiles per super-tile
F32 = mybir.dt.float32

# packed-constant column offsets (one [128, CST_COLS] DRAM tensor)
_CST_SPEC = [("w1t", 256), ("w2t", 64), ("w3t", 32), ("w4t", 8),
             ("sab1", NQ), ("sab2", NQ), ("gm", 72), ("cq", 8), ("cl", 8),
             ("rep", 72), ("red", 8), ("b1", 1), ("b2", 1), ("b3", 1),
             ("b4", 1), ("c0", 1)]
CO = {}
_off = 0
for _n, _w in _CST_SPEC:
    CO[_n] = _off
    _off += _w
CST_COLS = _off

_cache = {}


def _build_sindy_mats(sindy_w):
    """Derive the selection/coefficient matrices from sindy_w [9, 220]."""
    C = np.ascontiguousarray(np.asarray(sindy_w, np.float32)[:8])  # [8, 220]
    qlist = list(cwr(range(D), 2))
    tlist = list(cwr(range(D), 3))
    tidx = {t: i for i, t in enumerate(tlist)}
    c0 = C[:, 0].copy()
    CL = C[:, 1:1 + D].copy()                 # [8, 9]
    CQ = C[:, 1 + D:1 + D + NQ].copy()        # [8, 45]
    CC = C[:, 1 + D + NQ:].copy()             # [8, 165]

    SAB = np.zeros((90, D), np.float32)
    for q, (j, k) in enumerate(qlist):
        SAB[q, j] = 1.0
        SAB[45 + q, k] = 1.0
    A = np.zeros((72, NQ), np.float32)        # row 8*i+o
    for i in range(D):
        for q, (j, k) in enumerate(qlist):
            if j >= i:
                t = tidx[(i, j, k)]
                A[8 * i:8 * i + 8, q] = CC[:, t]
    REP = np.zeros((72, D), np.float32)
    RED = np.zeros((72, 8), np.float32)
    for i in range(D):
        for o in range(8):
            REP[8 * i + o, i] = 1.0
            RED[8 * i + o, o] = 1.0
    return dict(c0=c0, CL=CL, CQ=CQ, A=A, SAB=SAB, RED=RED, REP=REP)


def _build_graph():
    """Build the per-core Bass graph (same graph on all 8 cores)."""
    nc = bass.Bass()

    # --- DRAM parameters (order matters: inputs first as declared) ---
    x_d = nc.declare_dram_parameter("bin0", [BC, NBINS], F32, isOutput=False)
    m_d = nc.declare_dram_parameter("mrow", [1, BC], F32, isOutput=False)
    cst_d = nc.declare_dram_parameter("consts", [128, CST_COLS], F32,
                                      isOutput=False)
    out_d = nc.declare_dram_parameter("out", [8, BC], F32, isOutput=True)

    # view of bin0 for transposed-tile DMA: row = s*SUPER + c*128 + p
    x_view = x_d.rearrange("(s c p) f -> s p c f", s=NSUP, c=SUPER // 128, p=128)

    with tile.TileContext(nc) as tc, ExitStack() as ctx:
        const = ctx.enter_context(tc.tile_pool(name="const", bufs=1))
        xpool = ctx.enter_context(tc.tile_pool(name="x", bufs=3))
        spool = ctx.enter_context(tc.tile_pool(name="work", bufs=3))
        zpool = ctx.enter_context(tc.tile_pool(name="z9", bufs=2))
        opool = ctx.enter_context(tc.tile_pool(name="outs", bufs=2))
        pp = ctx.enter_context(tc.tile_pool(name="ps", bufs=1, space="PSUM"))
        ppx = ctx.enter_context(tc.tile_pool(name="psx", bufs=2, space="PSUM"))

        # all constants arrive in one DMA (single DMA-lane dependency)
        cst = const.tile([128, CST_COLS], F32)
        nc.sync.dma_start(out=cst, in_=cst_d[:, :])
        w1t = cst[:, CO["w1t"]:CO["w1t"] + 256].rearrange("p (c m) -> p c m", c=2)
        w2t = cst[:, CO["w2t"]:CO["w2t"] + 64]
        w3t = cst[0:64, CO["w3t"]:CO["w3t"] + 32]
        w4t = cst[0:32, CO["w4t"]:CO["w4t"] + 8]
        sab1 = cst[0:D, CO["sab1"]:CO["sab1"] + NQ]
        sab2 = cst[0:D, CO["sab2"]:CO["sab2"] + NQ]
        gm = cst[0:NQ, CO["gm"]:CO["gm"] + 72]
        cq = cst[0:NQ, CO["cq"]:CO["cq"] + 8]
        cl = cst[0:D, CO["cl"]:CO["cl"] + 8]
        rep = cst[0:D, CO["rep"]:CO["rep"] + 72]
        red = cst[0:72, CO["red"]:CO["red"] + 8]
        b1 = cst[:, CO["b1"]:CO["b1"] + 1]
        b2 = cst[0:64, CO["b2"]:CO["b2"] + 1]
        b3 = cst[0:32, CO["b3"]:CO["b3"] + 1]
        b4 = cst[0:8, CO["b4"]:CO["b4"] + 1]
        c0 = cst[0:8, CO["c0"]:CO["c0"] + 1]
        ident = const.tile([128, 128], F32)
        make_identity(nc, ident)

        # one-time dummy matmuls: absorb the gpsimd identity-build dep and the
        # const-DMA lane dep before the steady-state loop, so first real
        # matmuls carry at most one semaphore wait each
        warm_ps = pp.tile([128, TN], F32, tag="pz")
        nc.tensor.matmul(out=warm_ps[:, 0:128], lhsT=ident, rhs=ident,
                         start=True, stop=True)
        nc.tensor.matmul(out=warm_ps[:, 128:256], lhsT=cst[:, 0:128],
                         rhs=cst[:, 0:128], start=True, stop=True)
        warm_sb = spool.tile([8, 128], F32, tag="warm")
        nc.scalar.activation(out=warm_sb, in_=warm_ps[0:8, 0:128],
                             func=mybir.ActivationFunctionType.Copy)
        park_a = const.tile([1, 2 * NSUP], F32)
        park_b = const.tile([1, 2 * NSUP], F32)
        park_c = const.tile([1, 2 * NSUP], F32)

        prev_l1b = None
        prev_acts = None
        for s in range(NSUP):
            x_sb = xpool.tile([128, SUPER // 128, 256], F32, tag="x")
            last_pool = nc.gpsimd.dma_start(out=x_sb, in_=x_view[s])
            z9s = zpool.tile([D, SUPER], F32, tag="z9")
            nc.gpsimd.dma_start(out=z9s[8:9, :],
                                in_=m_d[0:1, s * SUPER:(s + 1) * SUPER])
            outs = opool.tile([8, SUPER], F32, tag="outs")
            # wait-parking ops: spare same-engine wait slots for the
            # legalizer to relocate buffer-recycling WAR waits onto
            # (persistent tiles: same-engine WAW needs no semaphore)
            pk1 = nc.scalar.activation(out=park_a[0:1, 2 * s:2 * s + 1],
                                       in_=cst[0:1, 0:1],
                                       func=mybir.ActivationFunctionType.Copy)
            pk2 = nc.scalar.activation(out=park_a[0:1, 2 * s + 1:2 * s + 2],
                                       in_=cst[0:1, 1:2],
                                       func=mybir.ActivationFunctionType.Copy)
            pk3 = nc.vector.tensor_copy(park_b[0:1, 2 * s:2 * s + 1],
                                        cst[0:1, 2:3])
            pk4 = nc.vector.tensor_copy(park_b[0:1, 2 * s + 1:2 * s + 2],
                                        cst[0:1, 3:4])
            pk5 = nc.gpsimd.memset(park_c[0:1, 2 * s:2 * s + 1], 0.0)
            pk6 = nc.gpsimd.memset(park_c[0:1, 2 * s + 1:2 * s + 2], 0.0)
            parks = (pk1, pk2, pk3, pk4, pk5, pk6)
            if prev_acts is not None:
                for pk in parks[:2]:
                    add_dep_helper(pk.ins, prev_acts[0].ins, False, "park-pos")
                for pk in parks[2:4]:
                    add_dep_helper(pk.ins, prev_acts[1].ins, False, "park-pos")
                for pk in parks[4:]:
                    add_dep_helper(pk.ins, prev_acts[2].ins, False, "park-pos")
            first_act = first_dve = None

            for t in range(NTILE):
                tsl = slice(t * TN, (t + 1) * TN)
                # --- transpose 512x256 input block to feature-major ---
                xt_ps0 = ppx.tile([128, TN], F32, tag="xt")
                xt_ps1 = ppx.tile([128, TN], F32, tag="xt")
                xt_ps = [xt_ps0, xt_ps1]
                first_pe = None
                if t == 0:
                    # 1-column dummies absorb the PSUM-slot WAR wait so the
                    # first real transposes only carry the x-DMA wait
                    first_pe = nc.tensor.matmul(
                        out=xt_ps0[:, 0:1], lhsT=ident,
                        rhs=ident[:, 0:1], start=True, stop=True)
                    nc.tensor.matmul(out=xt_ps1[:, 0:1], lhsT=ident,
                                     rhs=ident[:, 0:1], start=True, stop=True)
                    # absorb the M-row DMA wait before the SINDy matmuls
                    nc.tensor.matmul(out=xt_ps0[0:1, 1:2], lhsT=sab1[:, 0:1],
                                     rhs=z9s[:, 0:1], start=True, stop=True)
                for c in range(4):
                    for h in range(2):
                        mm = nc.tensor.matmul(
                            out=xt_ps[h][:, c * 128:(c + 1) * 128],
                            lhsT=x_sb[:, t * 4 + c, h * 128:(h + 1) * 128],
                            rhs=ident,
                            start=True, stop=True,
                        )
                        if first_pe is None:
                            first_pe = mm
                # keep PE program order tile-monotone: this tile's PE work
                # starts only after the previous tile's L1 (which carries the
                # DVE wait covering the xt-slot WARs)
                if prev_l1b is not None:
                    add_dep_helper(first_pe.ins, prev_l1b.ins, False, "tile-order")
                xt_sb0 = spool.tile([128, TN], F32, tag="xt_sb0")
                xt_sb1 = spool.tile([128, TN], F32, tag="xt_sb1")
                cpy0 = nc.vector.tensor_copy(xt_sb0, xt_ps[0])
                cpy1 = nc.scalar.activation(out=xt_sb1, in_=xt_ps[1],
                                            func=mybir.ActivationFunctionType.Copy)
                if first_act is None:
                    first_act, first_dve = cpy1, cpy0
                    for pk in parks[:2]:
                        add_dep_helper(first_act.ins, pk.ins, False, "park-order")
                    for pk in parks[2:]:
                        add_dep_helper(first_dve.ins, pk.ins, False, "park-order")
                # --- MLP ---
                a1_ps = pp.tile([128, TN], F32, tag="a1")
                nc.tensor.matmul(out=a1_ps, lhsT=w1t[:, 0, :], rhs=xt_sb0,
                                 start=True, stop=False)
                nc.tensor.matmul(out=a1_ps, lhsT=w1t[:, 1, :], rhs=xt_sb1,
                                 start=False, stop=True)
                a1_sb = spool.tile([128, TN], F32, tag="a1_sb")
                nc.scalar.activation(out=a1_sb, in_=a1_ps, bias=b1,
                                     func=mybir.ActivationFunctionType.Relu)
                a2_ps = pp.tile([64, TN], F32, tag="p64")
                nc.tensor.matmul(out=a2_ps, lhsT=w2t, rhs=a1_sb, start=True, stop=True)
                a2_sb = spool.tile([64, TN], F32, tag="a2_sb")
                nc.scalar.activation(out=a2_sb, in_=a2_ps, bias=b2,
                                     func=mybir.ActivationFunctionType.Relu)
                a3_ps = pp.tile([32, TN], F32, tag="p32")
                nc.tensor.matmul(out=a3_ps, lhsT=w3t, rhs=a2_sb, start=True, stop=True)
                a3_sb = spool.tile([32, TN], F32, tag="a3_sb")
                nc.scalar.activation(out=a3_sb, in_=a3_ps, bias=b3,
                                     func=mybir.ActivationFunctionType.Relu)
                z_ps = pp.tile([8, TN], F32, tag="pz")
                nc.tensor.matmul(out=z_ps, lhsT=w4t, rhs=a3_sb, start=True, stop=True)
                nc.scalar.activation(out=z9s[0:8, tsl], in_=z_ps, bias=b4,
                                     func=mybir.ActivationFunctionType.Identity)
                # --- SINDy ---
                f1_ps = pp.tile([NQ, TN], F32, tag="pf1")
                nc.tensor.matmul(out=f1_ps, lhsT=sab1, rhs=z9s[:, tsl],
                                 start=True, stop=True)
                f2_ps = pp.tile([NQ, TN], F32, tag="pf2")
                nc.tensor.matmul(out=f2_ps, lhsT=sab2, rhs=z9s[:, tsl],
                                 start=True, stop=True)
                f2_sb = spool.tile([NQ, TN], F32, tag="f2s")
                nc.vector.tensor_copy(f2_sb, f2_ps)
                p2_sb = spool.tile([NQ, TN], F32, tag="p2")
                nc.vector.tensor_mul(p2_sb, f1_ps, f2_sb)
                h_ps = pp.tile([72, TN], F32, tag="pf1")
                nc.tensor.matmul(out=h_ps, lhsT=gm, rhs=p2_sb, start=True, stop=True)
                zr_ps = pp.tile([72, TN], F32, tag="p64")
                nc.tensor.matmul(out=zr_ps, lhsT=rep, rhs=z9s[:, tsl],
                                 start=True, stop=True)
                zr_sb = spool.tile([72, TN], F32, tag="zrs")
                nc.vector.tensor_copy(zr_sb, zr_ps)
                zh_sb = spool.tile([72, TN], F32, tag="zh")
                last_dve = nc.vector.tensor_mul(zh_sb, zr_sb, h_ps)
                o_ps = pp.tile([8, TN], F32, tag="p32")
                nc.tensor.matmul(out=o_ps, lhsT=cq, rhs=p2_sb, start=True, stop=False)
                nc.tensor.matmul(out=o_ps, lhsT=cl, rhs=z9s[:, tsl],
                                 start=False, stop=False)
                prev_l1b = nc.tensor.matmul(out=o_ps, lhsT=red, rhs=zh_sb,
                                            start=False, stop=True)
                last_act = nc.scalar.activation(
                    out=outs[:, tsl], in_=o_ps, bias=c0,
                    func=mybir.ActivationFunctionType.Identity)
                prev_acts = (last_act, last_dve, last_pool)

            last_hw = nc.sync.dma_start(out=out_d[:, s * SUPER:(s + 1) * SUPER],
                                        in_=outs)

        # tail parking: SP nops after the final out-DMA give the end-of-kernel
        # drain (20 waits, budget 1) somewhere to spread its waits
        prev = last_hw
        for _k in range(24):
            tp = nc.sync.nop(hint="tailpark", nofuse=True)
            add_dep_helper(tp.ins, prev.ins, False, "tail-park")
            prev = tp

    _legalize_waits(nc)
    return nc


# walrus enforces small per-instruction sync-wait budgets (1 for Matmult's
# LDWEIGHTS half, 2 for ACT/DVE compute ops).  Tile emits redundant
# same-engine completion waits that push some instructions over.  Moving a
# wait to an EARLIER instruction on the same engine is always sound
# (program order preserves the guarantee); for same-engine waits it is also
# deadlock-free as long as the target's engine-tick exceeds the wait value.
_WAIT_LIMITS = {}  # every engine/DMA instruction: 1 wait
_COMPUTE_TYPES = {
    "InstDMACopy", "InstDrain", "InstNoOp",
    "InstMatmult", "InstLdweights", "InstActivation", "InstTensorTensor",
    "InstTensorCopy", "InstCopy", "InstTensorScalarAffineSelect",
    "InstMemset", "InstTensorReduce", "InstTensorScalar",
    "InstScalarTensorTensor", "InstIota", "InstTensorTensorScan",
}
_ENG_PREFIX = {
    "PE": "PE_", "Activation": "Activation_", "DVE": "DVE_",
    "Pool": "Pool_", "SP": "SP_",
}


def _legalize_waits(nc):
    moved_total = 0
    dropped = 0
    if True:
        insts = [i for b in nc.m.functions[0].blocks for i in b.instructions]
        n = len(insts)
        eng_of = [str(i.engine).split(".")[-1] for i in insts]
        # per-sem cumulative tick after each instruction + producer positions
        tick_after = [None] * n
        pos_of_tick = {}          # (sem, tick) -> position
        counters = {}
        for i, inst in enumerate(insts):
            si = inst.sync_info
            if si is not None and si.on_update:
                for u in si.on_update:
                    v = getattr(u, "value", 1) or 1
                    c0 = counters.get(u.ant_name, 0)
                    for k in range(c0 + 1, c0 + v + 1):
                        pos_of_tick[(u.ant_name, k)] = i
                    counters[u.ant_name] = c0 + v
            tick_after[i] = dict(counters)

        def producer_pos(sem, val):
            # position of the instruction whose update makes sem reach val.
            # DMA lanes increment by 16 per completed transfer on HW but by 1
            # in the BIR model, so convert wait values.
            if "DMAHW" in sem or "DMASW" in sem:
                val = (val + 15) // 16
            p = pos_of_tick.get((sem, val))
            return p if p is not None else 10 ** 9

        # completion requirements (sem -> tick) computed in trace order
        creq = [dict() for _ in range(n)]
        last_on_engine = {}
        for i, inst in enumerate(insts):
            r = {}
            j = last_on_engine.get(eng_of[i])
            if j is not None:
                r.update(creq[j])
            si = inst.sync_info
            if si is not None and si.on_wait:
                for w in si.on_wait:
                    r[w.ant_name] = max(r.get(w.ant_name, 0), w.wait_value)
                    p = producer_pos(w.ant_name, w.wait_value)
                    if p < n:
                        for s2, v2 in creq[p].items():
                            r[s2] = max(r.get(s2, 0), v2)
            creq[i] = r
            last_on_engine[eng_of[i]] = i

        def max_req_pos(req):
            mp = -1
            for s2, v2 in req.items():
                p = producer_pos(s2, v2)
                if p >= 10 ** 9:
                    return 10 ** 9
                mp = max(mp, p)
            return mp

        cur_waits = {}
        for i, inst in enumerate(insts):
            si = inst.sync_info
            cur_waits[i] = list(si.on_wait) if (si is not None and si.on_wait) else []

        for i, inst in enumerate(insts):
            nm = type(inst).__name__
            if nm not in _COMPUTE_TYPES:
                continue
            limit = _WAIT_LIMITS.get(nm, 1)
            waits = cur_waits[i]
            if len(waits) <= limit:
                continue
            eng = _ENG_PREFIX.get(eng_of[i], "???")
            # transitive elision: a wait implied by another kept wait's
            # producer chain (or by the same-engine predecessor) is redundant
            keep = list(waits)
            jprev = None
            for jj in range(i - 1, -1, -1):
                if eng_of[jj] == eng_of[i]:
                    jprev = jj
                    break
            changed = True
            while changed and len(keep) > limit:
                changed = False
                for w in list(keep):
                    implied = dict(creq[jprev]) if jprev is not None else {}
                    for w2 in keep:
                        if w2 is w:
                            continue
                        p2 = producer_pos(w2.ant_name, w2.wait_value)
                        if p2 < n:
                            for s2, v2 in creq[p2].items():
                                implied[s2] = max(implied.get(s2, 0), v2)
                    if implied.get(w.ant_name, 0) >= w.wait_value:
                        keep.remove(w)
                        dropped += 1
                        changed = True
                        break
            # prefer moving self-engine waits (always safe), then cross waits
            order = sorted(keep, key=lambda w: 0 if w.ant_name.startswith(eng) else 1)
            for w in order:
                if len(keep) <= limit:
                    break
                is_self = w.ant_name.startswith(eng)
                placed = False
                for j in range(i - 1, -1, -1):
                    pj = insts[j]
                    pnm = type(pj).__name__
                    if pnm not in _COMPUTE_TYPES or eng_of[j] != eng_of[i]:
                        continue
                    if len(cur_waits[j]) >= _WAIT_LIMITS.get(pnm, 1):
                        continue
                    if is_self:
                        prev = tick_after[j - 1].get(w.ant_name, 0) if j else 0
                        ok = prev >= w.wait_value
                        if not ok:
                            break
                    else:
                        p = producer_pos(w.ant_name, w.wait_value)
                        ok = (p < 10 ** 9
                              and max_req_pos(creq[p]) < j
                              and p != j)
                    if ok:
                        psi = pj.sync_info
                        if psi is None:
                            from concourse import mybir as _mb
                            pj.sync_info = _mb.SyncInfo(on_wait=[], on_update=[])
                            psi = pj.sync_info
                        cur_waits[j] = cur_waits[j] + [w]
                        psi.on_wait = cur_waits[j]
                        keep.remove(w)
                        placed = True
                        moved_total += 1
                        break
                if not placed:
                    if is_self and eng in ("DVE_", "Activation_", "Pool_"):
                        keep.remove(w)   # serial engine: hw order enforces it
                        dropped += 1
            if len(keep) > limit:
                raise RuntimeError(
                    f"cannot legalize {nm}@{i}: "
                    f"{[(x.ant_name, x.wait_value) for x in keep]}")
            inst.sync_info.on_wait = keep
            cur_waits[i] = keep
    print(f"_legalize_waits: moved {moved_total}, dropped {dropped}")


def _get_graph():
    if "nc" not in _cache:
        _cache["nc"] = _build_graph()
    return _cache["nc"]


def _make_in_maps(bin0, M, enc_w1, enc_b1, enc_w2, enc_b2, enc_w3, enc_b3,
                  enc_w4, enc_b4, sindy_w):
    mats = _build_sindy_mats(sindy_w)
    cst = np.zeros((128, CST_COLS), np.float32)

    def put(name, arr):
        arr = np.asarray(arr, np.float32)
        if arr.ndim == 1:
            arr = arr.reshape(-1, 1)
        r, c = arr.shape
        cst[:r, CO[name]:CO[name] + c] = arr

    w1t = np.asarray(enc_w1, np.float32).T           # [256, 128]
    put("w1t", np.concatenate([w1t[:128], w1t[128:]], axis=1))  # [128, 256]
    put("w2t", np.asarray(enc_w2, np.float32).T)
    put("w3t", np.asarray(enc_w3, np.float32).T)
    put("w4t", np.asarray(enc_w4, np.float32).T)
    put("sab1", mats["SAB"][:45].T)
    put("sab2", mats["SAB"][45:].T)
    put("gm", mats["A"].T)
    put("cq", mats["CQ"].T)
    put("cl", mats["CL"].T)
    put("rep", mats["REP"].T)
    put("red", mats["RED"])
    put("b1", enc_b1)
    put("b2", enc_b2)
    put("b3", enc_b3)
    put("b4", enc_b4)
    put("c0", mats["c0"])
    common = {"consts": cst}
    bin0 = np.asarray(bin0, np.float32)
    M = np.asarray(M, np.float32).reshape(-1)
    in_maps = []
    for i in range(N_CORES):
        sl = slice(i * BC, (i + 1) * BC)
        m = dict(common)
        m["bin0"] = np.ascontiguousarray(bin0[sl])
        m["mrow"] = np.ascontiguousarray(M[sl].reshape(1, BC))
        in_maps.append(m)
    return in_maps


def _ensure_ntff_hook():
    """The agent image lacks antenv.axon_hooks; synthesize it from
    trn_boot's ctypes NTFF hook so trace=True works under axon."""
    import sys
    import types
    if "antenv.axon_hooks" in sys.modules:
        return
    try:
        from trn_agent_boot.trn_boot import _ntff_profile_via_ctypes
        hook = _ntff_profile_via_ctypes("/opt/axon/libaxon_pjrt.so")
    except Exception:
        hook = None
    mod = types.ModuleType("antenv.axon_hooks")
    mod._hook = hook
    mod.get_axon_ntff_profile_hook = lambda: mod._hook
    mod.set_axon_ntff_profile_hook = lambda h: setattr(mod, "_hook", h)
    sys.modules["antenv.axon_hooks"] = mod


def run(trace=False, **inputs):
    if trace:
        _ensure_ntff_hook()
    nc = _get_graph()
    in_maps = _make_in_maps(**inputs)
    res = run_bass_kernel_spmd(nc, in_maps, core_ids=list(range(N_CORES)),
                               trace=trace)
    outs = []
    for i in range(N_CORES):
        o = np.asarray(res.results[i]["out"])  # [8, BC]
        outs.append(o.T)                       # [BC, 8]
    full = np.concatenate(outs, axis=0).reshape(B, 1, 8).astype(np.float32)
    return full, res


def kernel(**inputs) -> np.ndarray:
    out, _ = run(trace=False, **inputs)
    return out
